# revision 7
# baseline (speedup 1.0000x reference)
"""FlowNet Correlation (max_displacement=40) Trainium2 Bass kernel.

out[b, s, y, x] = sum_c x1[b,c,y,x] * x2p[b,c,y+dy,x+dx] / sqrt(C)
  with s = dy*81 + dx, dy,dx in [0,81), x2p zero-padded by 40 per side.

Strategy per core (shard over y: core k owns y in [8k, 8k+8), both batches):
  Pass 1: for each (b, y, dy-pair): band matmul rect[x, xp] =
     x1[:, y, :].T @ x2p[:, y+dy, :] (contraction over c=128), copy
     PSUM->SBUF, DMA the rectangle to a DRAM scratch tile.
  Pass 2: diagonal band extraction is a stride-(WP+1) access pattern in
     flat DRAM (a shear is un-expressible on-chip but trivial in DRAM):
     read band[x, dx] = rect[x, x+dx], PE-transpose to [dx, x], pack all
     dy into one SBUF tile, single strided DMA to the final layout.

Numerics: "hilo" mode splits each fp32 operand into bf16 hi + bf16 lo
and accumulates hi*hi + hi*lo + lo*hi into fp32 PSUM (3 chained
matmuls): ~2e-5 relative error at bf16 streaming speed. "f32r" mode is
a single matmul at FP22 precision (~1.5e-4 relative error).
"""

import math

import numpy as np

import concourse.bass as bass
import concourse.mybir as mybir
import concourse.tile as tile
from concourse import bacc
from concourse.bass_utils import run_bass_kernel_spmd
from concourse.masks import make_identity

F32 = mybir.dt.float32
F32R = mybir.dt.float32r
BF16 = mybir.dt.bfloat16

# Problem geometry (hardcoded per contract)
B, C, H, W, MD = 2, 128, 64, 96, 40
K = 2 * MD + 1            # 81
WP = W + 2 * MD           # 176
N_CORES = 8
YC = H // N_CORES         # 8 rows of y per core
HALO = YC + K - 1         # 88 rows of padded x2 per core

MODE = "hilo"             # "hilo" (bf16 hi+lo compensated) or "f32r"


def build_program(b_=B, c_=C, yc_=YC, w_=W, k_=K, dy_pack=2, mode=MODE):
    """Build the per-core Bass program. Geometry parameterized so a
    miniature version can be validated in CoreSim."""
    wp_ = w_ + k_ - 1
    halo_ = yc_ + k_ - 1
    k2 = k_ * k_

    nc = bacc.Bacc("TRN2", target_bir_lowering=False, debug=False, num_devices=8)
    in_dt = BF16 if mode == "hilo" else F32R
    names = ["h", "l"] if mode == "hilo" else [""]
    x1t = {
        s: nc.dram_tensor(f"x1{s}", [b_, c_, yc_, w_], in_dt, kind="ExternalInput")
        for s in names
    }
    x2t = {
        s: nc.dram_tensor(f"x2{s}", [b_, c_, halo_, wp_], in_dt, kind="ExternalInput")
        for s in names
    }
    out = nc.dram_tensor("out", [b_, k2, yc_, w_], F32, kind="ExternalOutput")

    n_pairs = k_ // dy_pack
    rem = k_ - n_pairs * dy_pack
    scr_sz = k_ * w_ * wp_

    with tile.TileContext(nc) as tc:
        with (
            tc.tile_pool(name="consts", bufs=1) as cpool,
            tc.tile_pool(name="x2pool", bufs=1) as x2pool,
            tc.tile_pool(name="x1pool", bufs=1) as x1pool,
            tc.tile_pool(name="stg", bufs=4) as stgpool,
            tc.tile_pool(name="shr", bufs=4) as shrpool,
            tc.tile_pool(name="fin", bufs=2) as finpool,
            tc.tile_pool(name="psA", bufs=4, space="PSUM") as psA,
            tc.tile_pool(name="psB", bufs=4, space="PSUM") as psB,
            tc.tile_pool(name="scrp", bufs=2, space="DRAM") as scrpool,
        ):
            ident = cpool.tile([128, 128], F32)
            make_identity(nc, ident[:])

            for b in range(b_):
                x2sb = {}
                for s in names:
                    x2sb[s] = x2pool.tile(
                        [c_, halo_ * wp_], in_dt, tag=f"x2sb{s}", name=f"x2sb{s}"
                    )
                    nc.sync.dma_start(
                        x2sb[s][:], x2t[s][b].rearrange("c h w -> c (h w)")
                    )
                x1sb = {}
                for s in names:
                    x1sb[s] = x1pool.tile(
                        [c_, yc_ * w_], in_dt, tag=f"x1sb{s}", name=f"x1sb{s}"
                    )
                    nc.sync.dma_start(
                        x1sb[s][:], x1t[s][b].rearrange("c h w -> c (h w)")
                    )

                for y in range(yc_):
                    scrt = scrpool.tile([scr_sz], F32, tag="scr", name="scrt")
                    ysl = slice(y * w_, (y + 1) * w_)

                    # ---- pass 1: band matmuls -> rect tiles -> scratch DRAM
                    groups = [(t * dy_pack, dy_pack) for t in range(n_pairs)]
                    if rem:
                        groups.append((n_pairs * dy_pack, rem))
                    for dy0, nd in groups:
                        nn_ = nd * wp_
                        ps = psA.tile([w_, dy_pack * wp_], F32, tag="ps", name="ps")
                        rsl = slice((y + dy0) * wp_, (y + dy0) * wp_ + nn_)
                        if mode == "hilo":
                            nc.tensor.matmul(
                                ps[:, :nn_], x1sb["h"][:, ysl], x2sb["h"][:, rsl],
                                start=True, stop=False,
                            )
                            nc.tensor.matmul(
                                ps[:, :nn_], x1sb["h"][:, ysl], x2sb["l"][:, rsl],
                                start=False, stop=False,
                            )
                            nc.tensor.matmul(
                                ps[:, :nn_], x1sb["l"][:, ysl], x2sb["h"][:, rsl],
                                start=False, stop=True,
                            )
                        else:
                            nc.tensor.matmul(
                                ps[:, :nn_], x1sb[""][:, ysl], x2sb[""][:, rsl],
                                start=True, stop=True,
                            )
                        st = stgpool.tile([w_, dy_pack * wp_], F32, tag="st", name="st")
                        nc.vector.tensor_copy(st[:, :nn_], ps[:, :nn_])
                        dst = bass.AP(
                            scrt.tensor,
                            scrt.offset + dy0 * w_ * wp_,
                            [[wp_, w_], [w_ * wp_, nd], [1, wp_]],
                        )
                        nc.sync.dma_start(
                            dst, st[:, :nn_].rearrange("p (d q) -> p d q", d=nd)
                        )

                    # ---- pass 2: sheared re-read + PE transpose + pack
                    outsb = finpool.tile([k_, k_ * w_], F32, tag="outsb", name="outsb")
                    grp = 3 if k_ % 3 == 0 else 1
                    for dy0 in range(0, k_, grp):
                        sh = shrpool.tile([w_, grp * k_], F32, tag="sh", name="sh")
                        src = bass.AP(
                            scrt.tensor,
                            scrt.offset + dy0 * w_ * wp_,
                            [[wp_ + 1, w_], [w_ * wp_, grp], [1, k_]],
                        )
                        nc.sync.dma_start(
                            sh[:].rearrange("p (g q) -> p g q", g=grp), src
                        )
                        for j in range(grp):
                            dy = dy0 + j
                            pst = psB.tile([k_, w_], F32, tag="pst", name="pst")
                            nc.tensor.transpose(
                                pst[:], sh[:, j * k_ : (j + 1) * k_], ident[:w_, :w_]
                            )
                            nc.vector.tensor_copy(
                                outsb[:, dy * w_ : (dy + 1) * w_], pst[:]
                            )

                    # ---- final strided store: partition=dx, runs along x
                    dst = bass.AP(
                        out,
                        b * k2 * yc_ * w_ + y * w_,
                        [[yc_ * w_, k_], [k_ * yc_ * w_, k_], [1, w_]],
                    )
                    nc.sync.dma_start(
                        dst, outsb[:].rearrange("p (d q) -> p d q", d=k_)
                    )
    nc.compile()
    return nc


_PROGRAM_CACHE = {}


def _get_program():
    if "full" not in _PROGRAM_CACHE:
        _PROGRAM_CACHE["full"] = build_program()
    return _PROGRAM_CACHE["full"]


def _split_hilo(a):
    import ml_dtypes

    hi = a.astype(ml_dtypes.bfloat16)
    lo = (a - hi.astype(np.float32)).astype(ml_dtypes.bfloat16)
    return hi, lo


def kernel(x1: np.ndarray, x2: np.ndarray) -> np.ndarray:
    x1 = np.ascontiguousarray(np.asarray(x1, dtype=np.float32))
    x2 = np.ascontiguousarray(np.asarray(x2, dtype=np.float32))

    # fold the 1/sqrt(C) normalization into x1 (free on host, 6 MB)
    x1n = x1 / np.float32(math.sqrt(C))
    x2p = np.pad(x2, ((0, 0), (0, 0), (MD, MD), (MD, MD)))

    if MODE == "hilo":
        x1h, x1l = _split_hilo(x1n)
        x2h, x2l = _split_hilo(x2p)
        srcs = {"x1h": x1h, "x1l": x1l, "x2h": x2h, "x2l": x2l}
    else:
        srcs = {"x1": x1n, "x2": x2p}

    in_maps = []
    for k in range(N_CORES):
        y0 = k * YC
        m = {}
        for name, arr in srcs.items():
            if name.startswith("x1"):
                m[name] = np.ascontiguousarray(arr[:, :, y0 : y0 + YC, :])
            else:
                m[name] = np.ascontiguousarray(arr[:, :, y0 : y0 + HALO, :])
        in_maps.append(m)

    nc = _get_program()
    res = run_bass_kernel_spmd(nc, in_maps, core_ids=list(range(N_CORES)))

    full = np.empty((B, K * K, H, W), dtype=np.float32)
    for k in range(N_CORES):
        full[:, :, k * YC : (k + 1) * YC, :] = res.results[k]["out"]
    return full


if __name__ == "__main__":
    from reference import reference, setup_inputs

    inputs = {k: np.asarray(v) for k, v in setup_inputs().items()}
    expected = np.asarray(reference(**inputs))
    actual = kernel(**inputs)
    err = np.abs(actual - expected).max() / np.abs(expected).max()
    print("Relative error:", err)
